# revision 1
# baseline (speedup 1.0000x reference)
"""Conv2D-KAN Trainium2 kernel (8-core data-parallel SPMD).

Formulation
-----------
The reference computes, per 3x3 patch (N = B*30*30 patches, in_size = 288):
    out[n,o] = sum_{i,k} sb[n,i,k] * (spline_kernel*scale)[i,k,o]
             + silu(xf) @ scale_factor + biases
where sb is a cubic B-spline basis (8 funcs) over a uniform grid
(knots t_r = -2.2 + 0.4 r, r = 0..11).

Key identities used here:
 1. Basis values depend only on the underlying *pixel*, not the patch
    (patch extraction is a gather), so features are computed per pixel
    (8x less elementwise work).
 2. Uniform cubic B-splines decompose over truncated powers:
        B_k(x) = sum_{m=0..4} c_m T_{k+m}(x),   c = [1,-4,6,-4,1]/6
        T_r(x) = relu((x - t_r)/h)^3
    so the (basis x weights) contraction folds into precomputed weights
    W'[r,...] = sum_k w[k,...] c_{r-k} and the features become 11 shifted
    clamped relu-cubes (T_11 is identically 0 after clamping).  Clamping
    a_r = min(relu(.), 11-r) makes the basis *exactly* zero for x beyond
    the grid (integer cancellation), matching the reference's
    out-of-range behaviour without masks.
 3. With 11 spline features + 1 silu feature per channel, the whole op
    is a 3x3 conv with 384 input channels and 128 filters: 27
    accumulating 128x128-weight matmuls per PSUM bank.

Each core processes 4 images: features [3 x (128 part, 4096 pix)] built
by ACT (affine relu / silu) + DVE (clamp, square, cube), then 8 PSUM
banks of [128 filters, 450 patches] accumulated over 27 matmuls each.
Output is written as [128, 3600] per core and transposed on host.
"""

import sys

sys.path.insert(0, "/opt/trn_rl_repo")

import numpy as np

N_CORES = 8
B, HH, WW, C = 32, 32, 32, 32
F = 128
KH = KW = 3
HO, WO = HH - KH + 1, WW - KW + 1          # 30, 30
BPC = B // N_CORES                          # images per core = 4
PIX_IM = HH * WW                            # 1024 pixels per image
NPC = BPC * HO * WO                         # 3600 patches per core
NBANK = 2 * BPC                             # 8 psum banks
BANKN = NPC // NBANK                        # 450
HGRID = 0.4
T0 = -2.2                                   # first knot
NR = 11                                     # truncated-cube features
NFEAT = 12                                  # + silu
CHUNKS = (NFEAT * C) // 128                 # 3 matmul K-chunks of 128
NMM = KH * KW * CHUNKS                      # 27 matmuls per bank

_cache = {}


def _build_program():
    import concourse.bacc as bacc
    import concourse.mybir as mybir
    import concourse.tile as tile

    f32 = mybir.dt.float32
    AF = mybir.ActivationFunctionType

    nc = bacc.Bacc("TRN2", target_bir_lowering=False, debug=False)
    xt = nc.dram_tensor("xt", [C, BPC * PIX_IM], f32, kind="ExternalInput").ap()
    wt = nc.dram_tensor("wt", [NMM, 128, F], f32, kind="ExternalInput").ap()
    consts = nc.dram_tensor("consts", [128, 8], f32, kind="ExternalInput").ap()
    y = nc.dram_tensor("y", [F, NPC], f32, kind="ExternalOutput").ap()

    with tile.TileContext(nc) as tc:
        with (
            tc.tile_pool(name="wp", bufs=1) as wp,
            tc.tile_pool(name="cp", bufs=1) as cp,
            tc.tile_pool(name="fp", bufs=3) as fp,
            tc.tile_pool(name="sp", bufs=3) as sp,
            tc.tile_pool(name="op", bufs=1) as op_,
            tc.tile_pool(name="pp", bufs=1, space="PSUM") as pp,
        ):
            ct = cp.tile([128, 8], f32)
            nc.sync.dma_start(ct[:], consts[:])

            wtiles = []
            for i in range(NMM):
                w = wp.tile([128, F], f32, tag=f"w{i}")
                nc.sync.dma_start(w[:], wt[i])
                wtiles.append(w)

            out_t = op_.tile([F, NPC], f32)

            for im in range(BPC):
                views = []
                for t in range(CHUNKS):
                    ft = fp.tile([128, PIX_IM], f32, tag=f"f{t}")
                    src = xt[:, im * PIX_IM:(im + 1) * PIX_IM]
                    for rep in range(4):
                        nc.sync.dma_start(ft[32 * rep:32 * rep + 32], src)
                    nsp = 128 if t < CHUNKS - 1 else 96
                    # a = relu(x/h + (5.5 - r))   [per-partition bias]
                    nc.scalar.activation(
                        ft[:nsp], ft[:nsp], AF.Relu,
                        bias=ct[:nsp, t:t + 1], scale=1.0 / HGRID,
                    )
                    if t == CHUNKS - 1:
                        nc.scalar.activation(ft[96:128], ft[96:128], AF.Silu)
                    # a = min(a, 11 - r); feature = a^3
                    nc.vector.tensor_scalar_min(
                        ft[:nsp], ft[:nsp], ct[:nsp, 3 + t:4 + t]
                    )
                    sq = sp.tile([128, PIX_IM], f32, tag="sq")
                    nc.vector.tensor_mul(sq[:nsp], ft[:nsp], ft[:nsp])
                    nc.vector.tensor_mul(ft[:nsp], sq[:nsp], ft[:nsp])
                    views.append(ft[:].rearrange("p (h w) -> p h w", w=WW))

                for half in range(2):
                    ps = pp.tile([F, BANKN], f32, tag=f"ps{im}_{half}")
                    k = 0
                    for off in range(KH * KW):
                        di, dj = divmod(off, KW)
                        h0 = half * 15 + di
                        for t in range(CHUNKS):
                            rhs = views[t][:, h0:h0 + 15, dj:dj + WO]
                            nc.tensor.matmul(
                                ps[:], wtiles[off * CHUNKS + t][:], rhs,
                                start=(k == 0), stop=(k == NMM - 1),
                            )
                            k += 1
                    s = (im * 2 + half) * BANKN
                    nc.scalar.activation(
                        out_t[:, s:s + BANKN], ps[:], AF.Identity,
                        bias=ct[:, 6:7], scale=1.0,
                    )

            nc.sync.dma_start(y[:], out_t[:])

    nc.compile()
    return nc


def _prep_static(spline_kernel, scale_factor, kan_bias, conv_bias):
    """Fold spline blending coefficients into conv weights (host, float64)."""
    w = spline_kernel.astype(np.float64) * scale_factor.astype(np.float64)[:, None, :]
    cm = np.array([1.0, -4.0, 6.0, -4.0, 1.0], np.float64) / 6.0
    # Wp[off, r, c, o]
    Wp = np.zeros((KH * KW, NFEAT, C, F), np.float64)
    wr = w.reshape(KH * KW, C, 8, F)
    for r in range(NR):
        for m in range(5):
            k = r - m
            if 0 <= k < 8:
                Wp[:, r] += wr[:, :, k] * cm[m]
    Wp[:, NR] = scale_factor.astype(np.float64).reshape(KH * KW, C, F)
    wt = Wp.reshape(KH * KW, NFEAT * C, F).reshape(KH * KW * CHUNKS, 128, F)
    wt = np.ascontiguousarray(wt, np.float32)

    consts = np.zeros((128, 8), np.float32)
    p = np.arange(128)
    for t in range(CHUNKS):
        r = 4 * t + p // 32
        consts[:, t] = -(T0 + HGRID * r) / HGRID          # 5.5 - r
        consts[:, 3 + t] = NR - r                          # 11 - r
    consts[:, 6] = (kan_bias.astype(np.float64)
                    + conv_bias.astype(np.float64)).astype(np.float32)
    return wt, consts


def kernel(x, spline_kernel, scale_factor, kan_bias, conv_bias):
    from concourse import bass_utils

    if "nc" not in _cache:
        _cache["nc"] = _build_program()
    nc = _cache["nc"]

    wt, consts = _prep_static(spline_kernel, scale_factor, kan_bias, conv_bias)

    in_maps = []
    for c in range(N_CORES):
        xc = x[c * BPC:(c + 1) * BPC]                      # (4,32,32,32)
        xtc = np.ascontiguousarray(
            xc.transpose(3, 0, 1, 2).reshape(C, BPC * PIX_IM), np.float32
        )
        in_maps.append({"xt": xtc, "wt": wt, "consts": consts})

    res = bass_utils.run_bass_kernel_spmd(
        nc, in_maps, core_ids=list(range(N_CORES)), **_cache.get("run_kwargs", {})
    )
    _cache["last_result"] = res

    out = np.empty((B, HO, WO, F), np.float32)
    for c in range(N_CORES):
        yc = res.results[c]["y"]                           # (128, 3600)
        out[c * BPC:(c + 1) * BPC] = (
            yc.reshape(F, BPC, HO, WO).transpose(1, 2, 3, 0)
        )
    return out


# revision 5
# speedup vs baseline: 1.7235x; 1.7235x over previous
"""Conv2D-KAN Trainium2 kernel (8-core data-parallel SPMD).

Formulation
-----------
The reference computes, per 3x3 patch (N = B*30*30 patches, in_size = 288):
    out[n,o] = sum_{i,k} sb[n,i,k] * (spline_kernel*scale)[i,k,o]
             + silu(xf) @ scale_factor + biases
where sb is a cubic B-spline basis (8 funcs) over a uniform grid
(knots t_r = -2.2 + 0.4 r, r = 0..11).

Key identities used here:
 1. Basis values depend only on the underlying *pixel*, not the patch
    (patch extraction is a gather), so features are computed per pixel
    (8x less elementwise work).
 2. Uniform cubic B-splines decompose over truncated powers:
        B_k(x) = sum_{m=0..4} c_m T_{k+m}(x),   c = [1,-4,6,-4,1]/6
        T_r(x) = relu((x - t_r)/h)^3
    so the (basis x weights) contraction folds into precomputed weights
    W'[r,...] = sum_k w[k,...] c_{r-k} and the features become 11 shifted
    clamped relu-cubes (T_11 is identically 0 after clamping).  Clamping
    a_r = min(relu(.), 11-r) makes the basis *exactly* zero for x beyond
    the grid (integer cancellation), matching the reference's
    out-of-range behaviour without masks.
 3. With 11 spline features + 1 silu feature per channel, the whole op
    is a 3x3 conv with 384 input channels and 128 filters: 27
    accumulating 128x128-weight matmuls per PSUM bank.

Each core processes 4 images: features [3 x (128 part, 4096 pix)] built
by ACT (affine relu / silu) + DVE (clamp, square, cube), then 8 PSUM
banks of [128 filters, 450 patches] accumulated over 27 matmuls each.
Output is written as [128, 3600] per core and transposed on host.
"""

import sys

sys.path.insert(0, "/opt/trn_rl_repo")

import numpy as np

N_CORES = 8
B, HH, WW, C = 32, 32, 32, 32
F = 128
KH = KW = 3
HO, WO = HH - KH + 1, WW - KW + 1          # 30, 30
BPC = B // N_CORES                          # images per core = 4
PIX_IM = HH * WW                            # 1024 pixels per image
NPC = BPC * HO * WO                         # 3600 patches per core
NBANK = 2 * BPC                             # 8 psum banks
BANKN = NPC // NBANK                        # 450
HGRID = 0.4
T0 = -2.2                                   # first knot
NR = 11                                     # truncated-cube features
NFEAT = 12                                  # + silu
CHUNKS = (NFEAT * C) // 128                 # 3 matmul K-chunks of 128
NMM = KH * KW * CHUNKS                      # 27 matmuls per bank

MODE = "fp32"  # "fp32" | "f32r"

_cache = {}


def _build_program(mode=None):
    import concourse.bacc as bacc
    import concourse.mybir as mybir
    import concourse.tile as tile

    mode = mode or MODE
    f32 = mybir.dt.float32
    mmdt = mybir.dt.float32r if mode == "f32r" else f32
    AF = mybir.ActivationFunctionType

    nc = bacc.Bacc("TRN2", target_bir_lowering=False, debug=False)
    xt = nc.dram_tensor("xt", [C, BPC * PIX_IM], f32, kind="ExternalInput").ap()
    wt = nc.dram_tensor("wt", [NMM, 128, F], f32, kind="ExternalInput").ap()
    consts = nc.dram_tensor("consts", [128, 8], f32, kind="ExternalInput").ap()
    y = nc.dram_tensor("y", [F, NPC], f32, kind="ExternalOutput").ap()

    with tile.TileContext(nc) as tc:
        with (
            tc.tile_pool(name="wp", bufs=1) as wp,
            tc.tile_pool(name="cp", bufs=1) as cp,
            tc.tile_pool(name="fp", bufs=3) as fp,
            tc.tile_pool(name="sp", bufs=3) as sp,
            tc.tile_pool(name="op", bufs=1) as op_,
            tc.tile_pool(name="pp", bufs=1, space="PSUM") as pp,
        ):
            ct = cp.tile([128, 8], f32)
            nc.sync.dma_start(ct[:], consts[:])

            wtiles = []
            for i in range(NMM):
                w = wp.tile([128, F], f32, tag=f"w{i}")
                nc.sync.dma_start(w[:], wt[i])
                if mode == "f32r":
                    wr = wp.tile([128, F], mmdt, tag=f"wr{i}")
                    nc.vector.tensor_copy(wr[:], w[:])
                    w = wr
                wtiles.append(w)

            out_t = op_.tile([F, NPC], f32)

            for im in range(BPC):
                views = []
                for t in range(CHUNKS):
                    ft = fp.tile([128, PIX_IM], f32, tag=f"f{t}")
                    src = xt[:, im * PIX_IM:(im + 1) * PIX_IM]
                    for rep in range(4):
                        nc.sync.dma_start(ft[32 * rep:32 * rep + 32], src)
                    nsp = 128 if t < CHUNKS - 1 else 96
                    # a = relu(x/h + (5.5 - r))   [per-partition bias]
                    nc.scalar.activation(
                        ft[:nsp], ft[:nsp], AF.Relu,
                        bias=ct[:nsp, t:t + 1], scale=1.0 / HGRID,
                    )
                    if t == CHUNKS - 1:
                        nc.scalar.activation(ft[96:128], ft[96:128], AF.Silu)
                    # a = min(a, 11 - r); feature = a^3
                    nc.vector.tensor_scalar_min(
                        ft[:nsp], ft[:nsp], ct[:nsp, 3 + t:4 + t]
                    )
                    sq = sp.tile([128, PIX_IM], f32, tag="sq")
                    nc.vector.tensor_mul(sq[:nsp], ft[:nsp], ft[:nsp])
                    if mode == "f32r":
                        fr = fp.tile([128, PIX_IM], mmdt, tag=f"fr{t}")
                        nc.vector.tensor_mul(fr[:nsp], sq[:nsp], ft[:nsp])
                        if t == CHUNKS - 1:
                            nc.vector.tensor_copy(fr[96:128], ft[96:128])
                        ft = fr
                    else:
                        nc.vector.tensor_mul(ft[:nsp], sq[:nsp], ft[:nsp])
                    views.append(ft[:].rearrange("p (h w) -> p h w", w=WW))

                for half in range(2):
                    ps = pp.tile([F, BANKN], f32, tag=f"ps{im}_{half}")
                    k = 0
                    for off in range(KH * KW):
                        di, dj = divmod(off, KW)
                        h0 = half * 15 + di
                        for t in range(CHUNKS):
                            rhs = views[t][:, h0:h0 + 15, dj:dj + WO]
                            nc.tensor.matmul(
                                ps[:], wtiles[off * CHUNKS + t][:], rhs,
                                start=(k == 0), stop=(k == NMM - 1),
                            )
                            k += 1
                    s = (im * 2 + half) * BANKN
                    nc.scalar.activation(
                        out_t[:, s:s + BANKN], ps[:], AF.Identity,
                        bias=ct[:, 6:7], scale=1.0,
                    )

            nc.sync.dma_start(y[:], out_t[:])

    nc.compile()
    return nc


def _prep_static(spline_kernel, scale_factor, kan_bias, conv_bias):
    """Fold spline blending coefficients into conv weights (host, float64)."""
    w = spline_kernel.astype(np.float64) * scale_factor.astype(np.float64)[:, None, :]
    cm = np.array([1.0, -4.0, 6.0, -4.0, 1.0], np.float64) / 6.0
    # Wp[off, r, c, o]
    Wp = np.zeros((KH * KW, NFEAT, C, F), np.float64)
    wr = w.reshape(KH * KW, C, 8, F)
    for r in range(NR):
        for m in range(5):
            k = r - m
            if 0 <= k < 8:
                Wp[:, r] += wr[:, :, k] * cm[m]
    Wp[:, NR] = scale_factor.astype(np.float64).reshape(KH * KW, C, F)
    wt = Wp.reshape(KH * KW, NFEAT * C, F).reshape(KH * KW * CHUNKS, 128, F)
    wt = np.ascontiguousarray(wt, np.float32)

    consts = np.zeros((128, 8), np.float32)
    p = np.arange(128)
    for t in range(CHUNKS):
        r = 4 * t + p // 32
        consts[:, t] = -(T0 + HGRID * r) / HGRID          # 5.5 - r
        consts[:, 3 + t] = NR - r                          # 11 - r
    consts[:, 6] = (kan_bias.astype(np.float64)
                    + conv_bias.astype(np.float64)).astype(np.float32)
    return wt, consts


def kernel(x, spline_kernel, scale_factor, kan_bias, conv_bias):
    from concourse import bass_utils

    key = f"nc_{MODE}"
    if key not in _cache:
        _cache[key] = _build_program(MODE)
    nc = _cache[key]

    wt, consts = _prep_static(spline_kernel, scale_factor, kan_bias, conv_bias)

    in_maps = []
    for c in range(N_CORES):
        xc = x[c * BPC:(c + 1) * BPC]                      # (4,32,32,32)
        xtc = np.ascontiguousarray(
            xc.transpose(3, 0, 1, 2).reshape(C, BPC * PIX_IM), np.float32
        )
        in_maps.append({"xt": xtc, "wt": wt, "consts": consts})

    res = bass_utils.run_bass_kernel_spmd(
        nc, in_maps, core_ids=list(range(N_CORES)), **_cache.get("run_kwargs", {})
    )
    _cache["last_result"] = res

    out = np.empty((B, HO, WO, F), np.float32)
    for c in range(N_CORES):
        yc = res.results[c]["y"]                           # (128, 3600)
        out[c * BPC:(c + 1) * BPC] = (
            yc.reshape(F, BPC, HO, WO).transpose(1, 2, 3, 0)
        )
    return out
